# revision 18
# baseline (speedup 1.0000x reference)
"""2-layer GCN (DGL GraphConv, norm='both') on 8 trn2 NeuronCores.

Strategy (v1: batched dma_gather):
  - Fold deg_out^-1/2 into x rows on host; apply deg_in^-1/2 (pre-relu) and the
    layer-2 source norm as per-partition activation scales on the Scalar engine.
  - Shard destination nodes across 8 cores (12544 rows each, 98 tiles of 128).
  - Edges sorted by (dst tile, src window); src windows of <=32768 rows so
    gather indices fit int16 for dma_gather. Per (tile, window) segments padded
    to 128-edge blocks (uniform across cores for SPMD).
  - Per group of ~10 tiles: 4 dma_gather calls (one per window) fetch all edge
    source rows into an SBUF slab; onehot matrices built 32 blocks per DVE op;
    aggregation via PSUM += onehot^T @ msgs on the tensor engine.
  - Between layers z = (relu-scaled h) @ W2 shards are AllGathered (bf16,
    padded to 128 cols so layer 2 reuses the same 256B-row gather tables).
"""
import sys
sys.path.insert(0, "/opt/trn_rl_repo")
import numpy as np

NCORES = 8
P = 128
NW = 4                     # gather source windows (int16 index reach)
WBASES = [0, 27904, 55808, 67584]
WCUTS = [27904, 55808, 83712]
WSIZE = 32768
GMAX = 128                 # max blocks per tile-group (slab = GMAX*128 cols)
GTILES = 8                 # max tiles per group (bounds live PSUM accumulators)

LAST_RESULT = None


def _layout(cnt_seg):
    """cnt_seg: [T, NW] max-over-core edge counts. Returns the global slot
    layout: per-group call list, per-tile segment list, block offsets."""
    T = cnt_seg.shape[0]
    nblkseg = -(-cnt_seg // P)          # ceil
    for t in range(T):
        if nblkseg[t].sum() == 0:
            nblkseg[t][0] = 1
    tile_blk = nblkseg.sum(axis=1)
    # greedy tile groups
    groups = []
    cur = []
    for t in range(T):
        if cur and (tile_blk[cur].sum() + tile_blk[t] > GMAX
                    or len(cur) >= GTILES):
            groups.append(cur)
            cur = []
        cur.append(t)
    if cur:
        groups.append(cur)
    # global layout; within each (group, window) call put the segment with the
    # largest pad last so its pad slots can be trimmed via trailing -1 idxs
    pad_seg = nblkseg * P - cnt_seg            # pad slots per segment
    seg_base = np.zeros((T, NW), np.int64)     # slot offset of each segment
    gmeta = []   # per group: (gblk0, gblk, calls=[(w, colb, nb, nreg)],
    #              tiles=[(t, [(colb, nb)])])
    blk0 = 0
    for tl in groups:
        gblk = int(tile_blk[tl].sum())
        calls = []
        tsegs = {t: [] for t in tl}
        colb = 0
        for w in range(NW):
            nb_w = int(nblkseg[tl, w].sum())
            if nb_w == 0:
                continue
            c0 = colb
            order = sorted(tl, key=lambda t: pad_seg[t, w])
            for t in order:
                nb = int(nblkseg[t, w])
                if nb:
                    seg_base[t, w] = (blk0 + colb) * P
                    tsegs[t].append((colb, nb))
                    colb += nb
            trail = int(pad_seg[order[-1], w]) if nblkseg[order[-1], w] else 0
            nreg = nb_w * P - trail
            if nreg < P:
                trail, nreg = 0, nb_w * P
            calls.append((w, c0, nb_w, nreg))
        gmeta.append((blk0, gblk, calls, [(t, tsegs[t]) for t in tl]))
        blk0 += gblk
    return nblkseg, seg_base, gmeta, blk0


def _build_program(T, NSH, NPAD, TOTBLK, gmeta, dt16, dtf32):
    from concourse import bass, bacc, mybir, tile

    nc = bacc.Bacc(None, num_devices=NCORES)
    xb = nc.declare_dram_parameter("xb", [NPAD, P], dt16, isOutput=False)
    gidx = nc.declare_dram_parameter("gidx", [P, TOTBLK * 8], mybir.dt.int16, isOutput=False)
    edstl = nc.declare_dram_parameter("edstl", [P, TOTBLK], dt16, isOutput=False)
    w1 = nc.declare_dram_parameter("w1", [P, P], dt16, isOutput=False)
    b1 = nc.declare_dram_parameter("b1", [P, 1], dtf32, isOutput=False)
    w2 = nc.declare_dram_parameter("w2", [P, 64], dt16, isOutput=False)
    b2r = nc.declare_dram_parameter("b2r", [P, 64], dtf32, isOutput=False)
    iden = nc.declare_dram_parameter("iden", [P, P], dt16, isOutput=False)
    iota = nc.declare_dram_parameter("iota", [P, P], dt16, isOutput=False)
    s2c = nc.declare_dram_parameter("s2c", [P, T], dtf32, isOutput=False)
    s3c = nc.declare_dram_parameter("s3c", [P, T], dtf32, isOutput=False)
    out = nc.declare_dram_parameter("out", [NSH, 64], dtf32, isOutput=True)

    zsh = nc.dram_tensor("zsh", [NSH, P], dt16, kind="Internal")
    zwide = nc.dram_tensor("zwide", [NPAD, P], dt16, kind="Internal",
                           addr_space="Shared")
    TT = tile.TileContext
    AF = mybir.ActivationFunctionType
    OP = mybir.AluOpType

    def build_layer(tc, cp, src_dram, layer):
        consts = {}
        for nm, dram, shp, dt in (
            ("gidx", gidx, [P, TOTBLK * 8], mybir.dt.int16),
            ("edstl", edstl, [P, TOTBLK], dt16),
            ("iota", iota, [P, P], dt16),
            ("s2c", s2c, [P, T], dtf32),
        ):
            t_ = cp.tile(shp, dt, tag=f"{nm}{layer}")
            nc.sync.dma_start(out=t_[:], in_=dram[:])
            consts[nm] = t_
        if layer == 1:
            for nm, dram, shp, dt in (
                ("w1", w1, [P, P], dt16), ("w2", w2, [P, 64], dt16),
                ("b1", b1, [P, 1], dtf32), ("iden", iden, [P, P], dt16),
                ("s3c", s3c, [P, T], dtf32),
            ):
                t_ = cp.tile(shp, dt, tag=f"{nm}{layer}")
                nc.sync.dma_start(out=t_[:], in_=dram[:])
                consts[nm] = t_
        else:
            t_ = cp.tile([P, 64], dtf32, tag=f"b2r{layer}")
            nc.sync.dma_start(out=t_[:], in_=b2r[:])
            consts["b2r"] = t_

        with (
            tc.tile_pool(name=f"slab{layer}", bufs=2) as slp,
            tc.tile_pool(name=f"oh{layer}", bufs=3) as ohp,
            tc.tile_pool(name=f"sb{layer}", bufs=3) as sp,
            tc.tile_pool(name=f"st{layer}", bufs=2) as stp,
            tc.tile_pool(name=f"pm{layer}", bufs=3, space="PSUM") as pmp,
            tc.tile_pool(name=f"pe{layer}", bufs=3, space="PSUM") as pep,
        ):
            for (blk0, gblk, calls, tsegs) in gmeta:
                fw = P if layer == 1 else 64
                slab = slp.tile([P, GMAX * P], dt16, tag="slab")
                for (w, colb, nb, nreg) in calls:
                    nc.gpsimd.dma_gather(
                        out_ap=slab[:, colb * P:(colb + nb) * P]
                            .rearrange("p (k n) -> p k n", n=P),
                        in_ap=src_dram[WBASES[w]:WBASES[w] + WSIZE, :],
                        idxs_ap=consts["gidx"][:, (blk0 + colb) * 8:(blk0 + colb + nb) * 8],
                        num_idxs=nb * P,
                        num_idxs_reg=nreg,
                        elem_size=P,
                        single_packet=False,
                    )
                oh = ohp.tile([P, GMAX * P], dt16, tag="oh")
                nc.vector.tensor_tensor(
                    out=oh[:, :gblk * P].rearrange("p (k n) -> p k n", n=P),
                    in0=consts["edstl"][:, blk0:blk0 + gblk]
                        .unsqueeze(2).broadcast_to([P, gblk, P]),
                    in1=consts["iota"][:].unsqueeze(1).broadcast_to([P, gblk, P]),
                    op=OP.is_equal,
                )
                g0 = tsegs[0][0]
                ng = len(tsegs)
                if layer == 1:
                    zst = stp.tile([P, GTILES * 64], dt16, tag="zst")
                else:
                    ost = stp.tile([P, GTILES * 64], dtf32, tag="ost")
                for ti, (t, segs) in enumerate(tsegs):
                    nseg = sum(nb for _, nb in segs)
                    psum = pmp.tile([P, fw], mybir.dt.float32, tag="pm")
                    done = 0
                    for (colb, nb) in segs:
                        for b in range(nb):
                            k = colb + b
                            nc.tensor.matmul(
                                out=psum[:],
                                lhsT=oh[:, k * P:(k + 1) * P],
                                rhs=slab[:, k * P:k * P + fw],
                                start=(done == 0), stop=(done == nseg - 1),
                            )
                            done += 1
                    if layer == 1:
                        msc = sp.tile([P, P], dt16, tag="msc")
                        nc.scalar.activation(out=msc[:], in_=psum[:],
                                             func=AF.Copy,
                                             scale=consts["s2c"][:, t:t + 1])
                        pt = pep.tile([P, P], mybir.dt.float32, tag="pe")
                        nc.tensor.matmul(out=pt[:], lhsT=msc[:],
                                         rhs=consts["iden"][:],
                                         start=True, stop=True)
                        mtt = sp.tile([P, P], dt16, tag="mtt")
                        nc.scalar.activation(out=mtt[:], in_=pt[:], func=AF.Copy)
                        ph = pep.tile([P, P], mybir.dt.float32, tag="pe")
                        nc.tensor.matmul(out=ph[:], lhsT=consts["w1"][:],
                                         rhs=mtt[:], start=True, stop=True)
                        ht = sp.tile([P, P], dt16, tag="ht")
                        nc.scalar.activation(out=ht[:], in_=ph[:], func=AF.Relu,
                                             bias=consts["b1"][:, :1], scale=1.0)
                        pz = pep.tile([P, 64], mybir.dt.float32, tag="pe")
                        nc.tensor.matmul(out=pz[:], lhsT=ht[:],
                                         rhs=consts["w2"][:, :64],
                                         start=True, stop=True)
                        nc.scalar.activation(out=zst[:, ti * 64:(ti + 1) * 64],
                                             in_=pz[:], func=AF.Copy,
                                             scale=consts["s3c"][:, t:t + 1])
                    else:
                        ot = sp.tile([P, 64], dtf32, tag="ot")
                        nc.scalar.activation(out=ot[:], in_=psum[:],
                                             func=AF.Copy,
                                             scale=consts["s2c"][:, t:t + 1])
                        nc.vector.tensor_tensor(
                            out=ost[:, ti * 64:(ti + 1) * 64],
                            in0=ot[:], in1=consts["b2r"][:], op=OP.add)
                if layer == 1:
                    nc.sync.dma_start(
                        out=zsh[g0 * P:(g0 + ng) * P, 0:64]
                            .rearrange("(t p) f -> p t f", p=P),
                        in_=zst[:, :ng * 64].rearrange("p (t f) -> p t f", f=64),
                    )
                else:
                    nc.sync.dma_start(
                        out=out[g0 * P:(g0 + ng) * P, :]
                            .rearrange("(t p) f -> p t f", p=P),
                        in_=ost[:, :ng * 64].rearrange("p (t f) -> p t f", f=64),
                    )

    with TT(nc) as tc:
        with tc.tile_pool(name="c1", bufs=1) as cp:
            build_layer(tc, cp, xb, 1)

    with nc.semaphore("cc_sem") as cc_sem:
        nc.gpsimd.collective_compute(
            "AllGather", mybir.AluOpType.bypass,
            replica_groups=[list(range(NCORES))],
            ins=[zsh[:]], outs=[zwide[:]],
        ).then_inc(cc_sem, 1)
        nc.sync.wait_ge(cc_sem, 1)
        nc.all_engine_barrier()

    with TT(nc) as tc:
        with tc.tile_pool(name="c2", bufs=1) as cp:
            build_layer(tc, cp, zwide, 2)

    nc.finalize()
    return nc


def kernel(in_feat, src, dst, W1, b1, W2, b2):
    global LAST_RESULT
    from concourse import mybir
    from concourse.bass_utils import run_bass_kernel_spmd

    in_feat = np.asarray(in_feat, np.float32)
    src = np.asarray(src, np.int32)
    dst = np.asarray(dst, np.int32)
    W1 = np.asarray(W1, np.float32)
    b1 = np.asarray(b1, np.float32)
    W2 = np.asarray(W2, np.float32)
    b2 = np.asarray(b2, np.float32)

    N, F = in_feat.shape
    O = W2.shape[1]
    assert F == P and W1.shape[1] == P and O == 64
    NPAD = int(np.ceil(N / (NCORES * P))) * NCORES * P   # 100352
    NSH = NPAD // NCORES                                  # 12544
    T = NSH // P                                          # 98

    deg_out = np.maximum(np.bincount(src, minlength=N), 1).astype(np.float32)
    deg_in = np.maximum(np.bincount(dst, minlength=N), 1).astype(np.float32)
    s1 = deg_out ** -0.5
    s2 = deg_in ** -0.5

    # per-edge window / tile coordinates
    w_e = np.digitize(src, WCUTS).astype(np.int64)        # 0..3
    idx16 = (src - np.array(WBASES, np.int64)[w_e]).astype(np.int16)
    core_e = dst // NSH
    dl = dst - core_e * NSH
    t_e = dl // P
    dloc = (dl % P).astype(np.float32)

    # per (core, tile, window) counts -> uniform segment sizes
    cnt = np.zeros((NCORES, T, NW), np.int64)
    np.add.at(cnt, (core_e, t_e, w_e), 1)
    cnt_max = cnt.max(axis=0)
    nblkseg, seg_base, gmeta, TOTBLK = _layout(cnt_max)
    TOTSLOT = TOTBLK * P

    # slot assignment per core
    seg_of_edge = (t_e * NW + w_e) + core_e * (T * NW)
    order = np.argsort(seg_of_edge, kind="stable")
    seg_sorted = seg_of_edge[order]
    uniq, starts_idx, counts = np.unique(seg_sorted, return_index=True,
                                         return_counts=True)
    rank = np.arange(len(src)) - np.repeat(starts_idx, counts)
    seg_base_full = np.broadcast_to(seg_base.reshape(1, T, NW),
                                    (NCORES, T, NW)).reshape(-1)
    slot = seg_base_full[seg_sorted] + rank

    gidx_all = np.zeros((NCORES, TOTSLOT), np.int16)
    edstl_all = np.full((NCORES, TOTSLOT), -1.0, np.float32)
    core_sorted = core_e[order]
    gidx_all[core_sorted, slot] = idx16[order]
    edstl_all[core_sorted, slot] = dloc[order]
    # trailing pad slots of each gather call are skipped via negative idx
    for (blk0g, gblk, calls, tsegs) in gmeta:
        for (w, c0, nb, nreg) in calls:
            s0 = (blk0g + c0) * P
            assert (gidx_all[:, s0 + nreg:s0 + nb * P] == 0).all()
            gidx_all[:, s0 + nreg:s0 + nb * P] = -1

    bf16 = mybir.dt.np(mybir.dt.bfloat16)
    xs = np.zeros((NPAD, P), np.float32)
    xs[:N] = in_feat * s1[:, None]
    xs = xs.astype(bf16)
    iota_np = np.tile(np.arange(P, dtype=np.float32), (P, 1)).astype(bf16)
    iden_np = np.eye(P, dtype=np.float32).astype(bf16)
    b1c = b1.reshape(P, 1).astype(np.float32)
    b2r_np = np.tile(b2.reshape(1, 64), (P, 1)).astype(np.float32)

    s2p = np.ones(NPAD, np.float32)
    s2p[:N] = s2
    s3p = np.ones(NPAD, np.float32)
    s3p[:N] = s1

    nc = _build_program(T, NSH, NPAD, TOTBLK, gmeta,
                        mybir.dt.bfloat16, mybir.dt.float32)

    in_maps = []
    for c in range(NCORES):
        g16 = gidx_all[c].reshape(-1, 16).T              # [16, TOTSLOT/16]
        gp = np.tile(g16, (8, 1))                        # [128, TOTSLOT/16]
        ed = edstl_all[c].reshape(-1, P).T.astype(bf16)  # [128, TOTBLK]
        shard = slice(c * NSH, (c + 1) * NSH)
        in_maps.append({
            "xb": xs,
            "gidx": np.ascontiguousarray(gp),
            "edstl": np.ascontiguousarray(ed),
            "w1": W1.astype(bf16),
            "b1": b1c,
            "w2": W2.astype(bf16),
            "b2r": b2r_np,
            "iden": iden_np,
            "iota": iota_np,
            "s2c": np.ascontiguousarray(s2p[shard].reshape(T, P).T),
            "s3c": np.ascontiguousarray(s3p[shard].reshape(T, P).T),
        })

    res = run_bass_kernel_spmd(nc, in_maps, list(range(NCORES)))
    LAST_RESULT = res
    out_full = np.concatenate([res.results[c]["out"] for c in range(NCORES)],
                              axis=0)
    return out_full[:N].astype(np.float32)
